# revision 1
# baseline (speedup 1.0000x reference)
"""3-level db4 wavelet low/high split for (32, 64, 16384) fp32 on 8 TRN2 NeuronCores.

Math: the reference computes wavedec (3-level db4, symmetric padding), then two
waverecs: `low` (details zeroed) and `high` (approximation zeroed).  Wavelets
give perfect reconstruction, so low + high == x and only the lowpass path is
needed: low = G @ (H @ x_row) with H (2054 x 16384) the composite 3-level
lowpass analysis operator (symmetric extension folded in) and G (16384 x 2054)
the lowpass synthesis operator; high = x - low on-chip.

Both operators are banded and 8-periodic, so all their 128-wide tiles dedupe
to 11 (stage 1) + 3 (stage 2) distinct weight tiles (~1.5 MB).

Device pipeline per core (256 rows = 2 row-groups of 128 partitions):
 - PE transpose-mode: x_row [rows, pos] -> x_sig [pos, rows] per 128-block
 - stage 1 (PE, fp32r): a3_sig[ab] = sum_pb HT_tile(pb,ab).T @ x_sig[pb]
 - stage 2 (PE, fp32r): low_row = a3_sig-as-stationary @ GT_tile  -> row-major
 - DVE: high = x - low (PSUM read);  ACT: PSUM->SBUF copies
Sharding: batch*feature rows 2048 -> 256 rows per core, zero communication.
"""

import numpy as np
import scipy.sparse as sp

import concourse.bacc as bacc
import concourse.tile as tile
from concourse import mybir
from concourse.bass_utils import run_bass_kernel_spmd

F32 = mybir.dt.float32
F32R = mybir.dt.float32r

DEC_LO = np.array([-0.010597401785069032, 0.032883011666982945, 0.030841381835986965,
                   -0.18703481171888114, -0.02798376941698385, 0.6308807679295904,
                   0.7148465705525415, 0.23037781330885523], dtype=np.float64)
REC_LO = DEC_LO[::-1].copy()
F = 8
N_CORES = 8


def _symidx(n):
    idx = np.concatenate([np.arange(6, -1, -1), np.arange(n), np.arange(n - 1, n - 8, -1)])
    return idx[1:]


def _dwt_lo_mat(n):
    ext_idx = _symidx(n)
    lout = (n + 13 - F) // 2 + 1
    filt = DEC_LO[::-1]
    rows = np.repeat(np.arange(lout), F)
    cols = ext_idx[(2 * np.arange(lout)[:, None] + np.arange(F)[None, :]).ravel()]
    vals = np.tile(filt, lout)
    return sp.coo_matrix((vals, (rows, cols)), shape=(lout, n)).tocsr()


def _idwt_lo_mat(n):
    lout = 2 * n + 1 - F + 1
    filt = REC_LO[::-1]
    rows, cols, vals = [], [], []
    i = np.arange(lout)
    for k in range(F):
        pos = i + k
        m = (pos % 2 == 1)
        rows.append(i[m])
        cols.append((pos[m] - 1) // 2)
        vals.append(np.full(int(m.sum()), filt[k]))
    return sp.coo_matrix(
        (np.concatenate(vals), (np.concatenate(rows), np.concatenate(cols))),
        shape=(lout, n)).tocsr()


def _build_H_G(L, level=3):
    H = sp.identity(L, format="csr")
    lens = []
    n = L
    for _ in range(level):
        lens.append(n)
        D = _dwt_lo_mat(n)
        H = D @ H
        n = D.shape[0]
    G = sp.identity(n, format="csr")
    a_len = n
    for ln in lens[::-1]:
        d_len = (ln + F - 1) // 2
        if a_len == d_len + 1:
            G = sp.identity(a_len, format="csr")[:-1] @ G
            a_len -= 1
        U = _idwt_lo_mat(a_len)
        G = U @ G
        a_len = U.shape[0]
    return H, G


def _build_plan(L):
    H, G = _build_H_G(L)
    na = H.shape[0]
    nab = (na + 127) // 128
    HTp = np.zeros((L, nab * 128), np.float32)
    HTp[:, :na] = np.asarray(H.T.todense(), np.float32)
    GTp = np.zeros((nab * 128, L), np.float32)
    GTp[:na, :] = np.asarray(G.T.todense(), np.float32)

    npb = L // 128
    nw = L // 512

    s1_tiles, s1map = {}, [[] for _ in range(nab)]
    for ab in range(nab):
        for pb in range(npb):
            t = HTp[128 * pb:128 * pb + 128, 128 * ab:128 * ab + 128]
            if np.any(t):
                tid = s1_tiles.setdefault(t.tobytes(), len(s1_tiles))
                s1map[ab].append((pb, tid))
    w1 = np.zeros((128, 128 * len(s1_tiles)), np.float32)
    for key, tid in s1_tiles.items():
        w1[:, 128 * tid:128 * tid + 128] = np.frombuffer(key, np.float32).reshape(128, 128)

    s2_tiles, s2map = {}, [[] for _ in range(nw)]
    for ab in range(nab):
        for w in range(nw):
            t = GTp[128 * ab:128 * ab + 128, 512 * w:512 * w + 512]
            if np.any(t):
                nzc = np.nonzero(np.any(t != 0, axis=0))[0]
                tid = s2_tiles.setdefault(t.tobytes(), len(s2_tiles))
                s2map[w].append((ab, tid, int(nzc.min()), int(nzc.max()) + 1))
    for w in range(nw):
        s2map[w].sort(key=lambda e: -(e[3] - e[2]))  # widest (full-bank) first
    w2 = np.zeros((128, 512 * len(s2_tiles)), np.float32)
    for key, tid in s2_tiles.items():
        w2[:, 512 * tid:512 * tid + 512] = np.frombuffer(key, np.float32).reshape(128, 512)

    first_need = {}
    for ab in range(nab):
        for pb, _ in s1map[ab]:
            first_need.setdefault(pb, ab)
    tsched = [[] for _ in range(nab)]
    for pb, ab in first_need.items():
        tsched[ab].append(pb)
    wsched = [[] for _ in range(nab)]
    for w in range(nw):
        wsched[max(ab for ab, _, _, _ in s2map[w])].append(w)
    for lst in tsched:
        lst.sort()
    for lst in wsched:
        lst.sort()

    return dict(L=L, nab=nab, npb=npb, nw=nw, w1=w1, w2=w2,
                s1map=s1map, s2map=s2map, tsched=tsched, wsched=wsched)


def _build_program(plan, rows, chunk=1024, xsig_bufs=16, xchunk_bufs=7,
                   mm_dt=F32R, out_bufs=4):
    L, nab, nw = plan["L"], plan["nab"], plan["nw"]
    nrg = rows // 128
    assert rows % 128 == 0 and nrg == 2
    nch = L // chunk
    pb_per_chunk = chunk // 128

    nc = bacc.Bacc("TRN2", target_bir_lowering=False, debug=False)
    x_d = nc.dram_tensor("x", [rows, L], F32, kind="ExternalInput").ap()
    w1_d = nc.dram_tensor("w1", list(plan["w1"].shape), mm_dt, kind="ExternalInput").ap()
    w2_d = nc.dram_tensor("w2", list(plan["w2"].shape), mm_dt, kind="ExternalInput").ap()
    id_d = nc.dram_tensor("ident", [128, 128], F32, kind="ExternalInput").ap()
    low_d = nc.dram_tensor("low", [rows, L], F32, kind="ExternalOutput").ap()
    high_d = nc.dram_tensor("high", [rows, L], F32, kind="ExternalOutput").ap()

    with tile.TileContext(nc) as tc:
        with tc.tile_pool(name="sbw", bufs=1) as sbw, \
             tc.tile_pool(name="sbx", bufs=xchunk_bufs) as sbx, \
             tc.tile_pool(name="sbxs", bufs=xsig_bufs) as sbxs, \
             tc.tile_pool(name="sba3", bufs=nab) as sba3, \
             tc.tile_pool(name="sbo", bufs=out_bufs) as sbo, \
             tc.tile_pool(name="pst", bufs=3, space="PSUM") as pst, \
             tc.tile_pool(name="psa", bufs=1, space="PSUM") as psa, \
             tc.tile_pool(name="ps2", bufs=2, space="PSUM") as ps2p:

            idt = sbw.tile([128, 128], F32, tag="idt")
            nc.sync.dma_start(idt[:], id_d[:])
            # weights early, spread over 4 slices each so no single DMA queue
            # carries the whole 1.5 MB
            w1t = sbw.tile(list(plan["w1"].shape), mm_dt, tag="w1t")
            wq = plan["w1"].shape[1] // 4
            for i in range(4):
                nc.sync.dma_start(w1t[:, i * wq:(i + 1) * wq], w1_d[:, i * wq:(i + 1) * wq])
            w2t = sbw.tile(list(plan["w2"].shape), mm_dt, tag="w2t")
            wq2 = plan["w2"].shape[1] // 4
            for i in range(4):
                nc.sync.dma_start(w2t[:, i * wq2:(i + 1) * wq2], w2_d[:, i * wq2:(i + 1) * wq2])

            xch, xsig, a3 = {}, {}, {}
            chunks_issued = set()
            ncopy = 0

            def ensure_chunk(c):
                if c in chunks_issued or c >= nch:
                    return
                chunks_issued.add(c)
                for rg in range(nrg):
                    xt = sbx.tile([128, chunk], F32, tag=f"x{rg}")
                    nc.gpsimd.dma_start(
                        xt[:], x_d[rg * 128:(rg + 1) * 128, c * chunk:(c + 1) * chunk])
                    xch[(rg, c)] = xt

            for k in range(nab):
                for pb in plan["tsched"][k]:
                    c = pb // pb_per_chunk
                    ensure_chunk(c)
                    ensure_chunk(c + 1)
                    off = (pb % pb_per_chunk) * 128
                    pt = pst.tile([128, 128 * nrg], F32, tag="pt")
                    for rg in range(nrg):
                        nc.tensor.transpose(
                            pt[:, rg * 128:(rg + 1) * 128],
                            xch[(rg, c)][:, off:off + 128], idt[:])
                    xs = sbxs.tile([128, 128 * nrg], mm_dt, tag="xs")
                    if ncopy % 3 == 0:
                        nc.scalar.copy(xs[:], pt[:])
                    else:
                        nc.vector.tensor_copy(xs[:], pt[:])
                    ncopy += 1
                    xsig[pb] = xs

                pa = psa.tile([128, 128 * nrg], F32, tag="pa")
                ents = plan["s1map"][k]
                for i, (pb, tid) in enumerate(ents):
                    nc.tensor.matmul(
                        pa[:], w1t[:, 128 * tid:128 * tid + 128], xsig[pb][:],
                        start=(i == 0), stop=(i == len(ents) - 1))
                a3t = sba3.tile([128, 128 * nrg], mm_dt, tag="a3")
                nc.scalar.copy(a3t[:], pa[:])
                a3[k] = a3t

                for w in plan["wsched"][k]:
                    c = (512 * w) // chunk
                    off = (512 * w) % chunk
                    for rg in range(nrg):
                        po = ps2p.tile([128, 512], F32, tag=f"s2r{rg}")
                        ents2 = plan["s2map"][w]
                        for j, (ab, tid, clo, chi) in enumerate(ents2):
                            nc.tensor.matmul(
                                po[:, clo:chi],
                                a3[ab][:, rg * 128:(rg + 1) * 128],
                                w2t[:, 512 * tid + clo:512 * tid + chi],
                                start=(j == 0), stop=(j == len(ents2) - 1))
                        lo = sbo.tile([128, 512], F32, tag=f"lo{rg}")
                        nc.scalar.copy(lo[:], po[:])
                        hi = sbo.tile([128, 512], F32, tag=f"hi{rg}")
                        nc.vector.tensor_sub(
                            hi[:], xch[(rg, c)][:, off:off + 512], po[:])
                        nc.sync.dma_start(
                            low_d[rg * 128:(rg + 1) * 128, 512 * w:512 * w + 512], lo[:])
                        nc.sync.dma_start(
                            high_d[rg * 128:(rg + 1) * 128, 512 * w:512 * w + 512], hi[:])

    nc.compile()
    return nc


_CACHE = {}


def kernel(x):
    x = np.asarray(x)
    B, Fd, L = x.shape
    in_dtype = x.dtype
    xf = np.ascontiguousarray(x.astype(np.float32, copy=False).reshape(B * Fd, L))
    rows = (B * Fd) // N_CORES

    key = (rows, L)
    if key not in _CACHE:
        plan = _build_plan(L)
        nc = _build_program(plan, rows=rows)
        _CACHE[key] = (plan, nc)
    plan, nc = _CACHE[key]

    ident = np.eye(128, dtype=np.float32)
    in_maps = [{
        "x": np.ascontiguousarray(xf[c * rows:(c + 1) * rows]),
        "w1": plan["w1"], "w2": plan["w2"], "ident": ident,
    } for c in range(N_CORES)]

    res = run_bass_kernel_spmd(nc, in_maps, list(range(N_CORES)))
    low = np.concatenate([r["low"] for r in res.results], axis=0).reshape(B, Fd, L)
    high = np.concatenate([r["high"] for r in res.results], axis=0).reshape(B, Fd, L)
    return low.astype(in_dtype, copy=False), high.astype(in_dtype, copy=False)



# revision 5
# speedup vs baseline: 1.7443x; 1.7443x over previous
"""3-level db4 wavelet low/high split for (32, 64, 16384) fp32 on 8 TRN2 NeuronCores.

Math: the reference computes wavedec (3-level db4, symmetric padding) then two
waverecs: `low` (details zeroed) and `high` (approximation zeroed).  Wavelets
give perfect reconstruction, so low + high == x and only the lowpass path is
needed: low = G @ (H @ x_row) with H (2054 x 16384) the composite 3-level
lowpass analysis operator (symmetric extension folded in) and G (16384 x 2054)
the lowpass synthesis operator; high = x - low on-chip.

Layout/sharding: the signal axis L is sharded across the 8 cores (2048
positions each + 128-position halo).  The host uploads x TRANSPOSED into
"sig" layout [pos, rows] as bf16, so every on-device matmul streams the 2048
fused batch*feature rows as the moving operand and NO on-device transposes
are needed; outputs are written back in sig layout as bf16 and the host
re-transposes while unsharding.  Both operator stages are banded: per core
only 20 (stage 1) + 18 (stage 2) distinct 128x128 weight tiles are nonzero.

Device pipeline per core:
 - 18 x-tile DMAs [128 pos, 2048 rows] bf16 (4 KiB/partition lines)
 - stage 1 (PE, bf16): a3[t] += W1(t,pb)^T @ x[pb], accumulated in PSUM
   (4 banks of [128, 512] per a-tile), ACT-copied to SBUF bf16
 - stage 2 (PE, bf16): low[o] += W2(o,t)^T @ a3[t] in PSUM
 - ACT: low PSUM -> SBUF bf16; DVE: high = x - low(PSUM) -> bf16
 - per-o DMAs of low/high [128, 2048] bf16 back to HBM
"""

import numpy as np
import scipy.sparse as sp
import ml_dtypes

import concourse.bacc as bacc
import concourse.tile as tile
from concourse import mybir
from concourse.bass_utils import run_bass_kernel_spmd

F32 = mybir.dt.float32
BF16 = mybir.dt.bfloat16
BF16_NP = ml_dtypes.bfloat16

DEC_LO = np.array([-0.010597401785069032, 0.032883011666982945, 0.030841381835986965,
                   -0.18703481171888114, -0.02798376941698385, 0.6308807679295904,
                   0.7148465705525415, 0.23037781330885523], dtype=np.float64)
REC_LO = DEC_LO[::-1].copy()
F = 8
N_CORES = 8
L = 16384
ROWS = 2048          # fused B*F rows
S = L // N_CORES     # 2048 positions per core
NPB = 18             # local x tiles (128-position halo each side)
NT = 3               # local a3 tiles (384-value a window)
NO = 16              # local output tiles
A_OFF = -64          # a-window start, relative to 256*c
X_OFF = -128         # x-window start, relative to 2048*c
NCHUNK = 4           # 2048 rows in 4 PSUM-bank chunks of 512


def _symidx(n):
    idx = np.concatenate([np.arange(6, -1, -1), np.arange(n), np.arange(n - 1, n - 8, -1)])
    return idx[1:]


def _dwt_lo_mat(n):
    ext_idx = _symidx(n)
    lout = (n + 13 - F) // 2 + 1
    filt = DEC_LO[::-1]
    rows = np.repeat(np.arange(lout), F)
    cols = ext_idx[(2 * np.arange(lout)[:, None] + np.arange(F)[None, :]).ravel()]
    vals = np.tile(filt, lout)
    return sp.coo_matrix((vals, (rows, cols)), shape=(lout, n)).tocsr()


def _idwt_lo_mat(n):
    lout = 2 * n + 1 - F + 1
    filt = REC_LO[::-1]
    rows, cols, vals = [], [], []
    i = np.arange(lout)
    for k in range(F):
        pos = i + k
        m = (pos % 2 == 1)
        rows.append(i[m])
        cols.append((pos[m] - 1) // 2)
        vals.append(np.full(int(m.sum()), filt[k]))
    return sp.coo_matrix(
        (np.concatenate(vals), (np.concatenate(rows), np.concatenate(cols))),
        shape=(lout, n)).tocsr()


def _build_H_G(L, level=3):
    H = sp.identity(L, format="csr")
    lens = []
    n = L
    for _ in range(level):
        lens.append(n)
        D = _dwt_lo_mat(n)
        H = D @ H
        n = D.shape[0]
    G = sp.identity(n, format="csr")
    a_len = n
    for ln in lens[::-1]:
        d_len = (ln + F - 1) // 2
        if a_len == d_len + 1:
            G = sp.identity(a_len, format="csr")[:-1] @ G
            a_len -= 1
        U = _idwt_lo_mat(a_len)
        G = U @ G
        a_len = U.shape[0]
    return H, G


def _slice_pad(M, r0, r1, c0, c1):
    out = np.zeros((r1 - r0, c1 - c0), np.float32)
    rr0, rr1 = max(r0, 0), min(r1, M.shape[0])
    cc0, cc1 = max(c0, 0), min(c1, M.shape[1])
    if rr0 < rr1 and cc0 < cc1:
        out[rr0 - r0:rr1 - r0, cc0 - c0:cc1 - c0] = M[rr0:rr1, cc0:cc1]
    return out


def _build_plan():
    H, G = _build_H_G(L)
    HT = np.asarray(H.T.todense(), np.float32)   # [L, na]
    GT = np.asarray(G.T.todense(), np.float32)   # [na, L]

    s1_pairs, s2_pairs = set(), set()
    w1, w2 = {}, {}
    for c in range(N_CORES):
        xbase = 2048 * c + X_OFF
        abase = 256 * c + A_OFF
        for t in range(NT):
            a0 = abase + 128 * t
            for pb in range(NPB):
                p0 = xbase + 128 * pb
                tl = _slice_pad(HT, p0, p0 + 128, a0, a0 + 128)
                if np.any(tl):
                    s1_pairs.add((t, pb))
                    w1[(c, t, pb)] = tl
        for o in range(NO):
            i0 = 2048 * c + 128 * o
            for t in range(NT):
                a0 = abase + 128 * t
                tl = _slice_pad(GT, a0, a0 + 128, i0, i0 + 128)
                if np.any(tl):
                    s2_pairs.add((o, t))
                    w2[(c, o, t)] = tl
    s1_pairs = sorted(s1_pairs)
    s2_pairs = sorted(s2_pairs)

    # weight blobs per core, one 128x128 slot per pair (zeros where the core
    # has no tile); stage-1 slots keyed by (t, pb), stage-2 by (o, t)
    w1_blob = np.zeros((N_CORES, 128, 128 * len(s1_pairs)), BF16_NP)
    w2_blob = np.zeros((N_CORES, 128, 128 * len(s2_pairs)), BF16_NP)
    for c in range(N_CORES):
        for i, (t, pb) in enumerate(s1_pairs):
            tl = w1.get((c, t, pb))
            if tl is not None:
                w1_blob[c, :, 128 * i:128 * i + 128] = tl.astype(BF16_NP)
        for i, (o, t) in enumerate(s2_pairs):
            tl = w2.get((c, o, t))
            if tl is not None:
                w2_blob[c, :, 128 * i:128 * i + 128] = tl.astype(BF16_NP)

    s1groups = [[] for _ in range(NT)]          # t -> [(pb, slot)]
    for i, (t, pb) in enumerate(s1_pairs):
        s1groups[t].append((pb, i))
    s2groups = [[] for _ in range(NO)]          # o -> [(t, slot)]
    for i, (o, t) in enumerate(s2_pairs):
        s2groups[o].append((t, i))
    return dict(w1=w1_blob, w2=w2_blob, s1groups=s1groups, s2groups=s2groups,
                n1=len(s1_pairs), n2=len(s2_pairs))


def _build_program(plan):
    nc = bacc.Bacc("TRN2", target_bir_lowering=False, debug=False)
    x_d = nc.dram_tensor("x", [NPB * 128, ROWS], BF16, kind="ExternalInput").ap()
    w1_d = nc.dram_tensor("w1", [128, 128 * plan["n1"]], BF16, kind="ExternalInput").ap()
    w2_d = nc.dram_tensor("w2", [128, 128 * plan["n2"]], BF16, kind="ExternalInput").ap()
    low_d = nc.dram_tensor("low", [S, ROWS], BF16, kind="ExternalOutput").ap()
    high_d = nc.dram_tensor("high", [S, ROWS], BF16, kind="ExternalOutput").ap()

    CH = ROWS // NCHUNK  # 512

    with tile.TileContext(nc) as tc:
        with tc.tile_pool(name="sbw", bufs=1) as sbw, \
             tc.tile_pool(name="sbx", bufs=1) as sbx, \
             tc.tile_pool(name="sba3", bufs=1) as sba3, \
             tc.tile_pool(name="sbo", bufs=3) as sbo, \
             tc.tile_pool(name="psa", bufs=1, space="PSUM") as psa, \
             tc.tile_pool(name="ps2", bufs=1, space="PSUM") as ps2:

            w1t = sbw.tile([128, 128 * plan["n1"]], BF16, tag="w1t")
            q = (plan["n1"] * 128) // 4
            for i in range(4):
                nc.sync.dma_start(w1t[:, i * q:(i + 1) * q], w1_d[:, i * q:(i + 1) * q])
            w2t = sbw.tile([128, 128 * plan["n2"]], BF16, tag="w2t")
            q2 = (plan["n2"] * 128) // 2
            for i in range(2):
                nc.sync.dma_start(w2t[:, i * q2:(i + 1) * q2], w2_d[:, i * q2:(i + 1) * q2])

            xt = []
            for pb in range(NPB):
                t_ = sbx.tile([128, ROWS], BF16, tag=f"x{pb}")
                nc.sync.dma_start(t_[:], x_d[128 * pb:128 * pb + 128, :])
                xt.append(t_)

            # output tiles o grouped by the last a3 tile they need
            o_after_t = [[] for _ in range(NT)]
            for o in range(NO):
                o_after_t[max(t for t, _ in plan["s2groups"][o])].append(o)

            a3 = [None] * NT
            for t in range(NT):
                ents = plan["s1groups"][t]
                pa = [psa.tile([128, CH], F32, tag=f"pa{k}", name=f"pa{k}")
                      for k in range(NCHUNK)]
                for j, (pb, slot) in enumerate(ents):
                    for k in range(NCHUNK):
                        nc.tensor.matmul(
                            pa[k][:], w1t[:, 128 * slot:128 * slot + 128],
                            xt[pb][:, CH * k:CH * k + CH],
                            start=(j == 0), stop=(j == len(ents) - 1))
                a3t = sba3.tile([128, ROWS], BF16, tag=f"a3_{t}")
                for k in range(NCHUNK):
                    nc.scalar.copy(a3t[:, CH * k:CH * k + CH], pa[k][:])
                a3[t] = a3t

                for o in o_after_t[t]:
                    ents2 = plan["s2groups"][o]
                    po = [ps2.tile([128, CH], F32, tag=f"po{k}", name=f"po{k}")
                          for k in range(NCHUNK)]
                    for j, (t2, slot) in enumerate(ents2):
                        for k in range(NCHUNK):
                            nc.tensor.matmul(
                                po[k][:], w2t[:, 128 * slot:128 * slot + 128],
                                a3[t2][:, CH * k:CH * k + CH],
                                start=(j == 0), stop=(j == len(ents2) - 1))
                    lo = sbo.tile([128, ROWS], BF16, tag="lo")
                    hi = sbo.tile([128, ROWS], BF16, tag="hi")
                    for k in range(NCHUNK):
                        nc.scalar.copy(lo[:, CH * k:CH * k + CH], po[k][:])
                        nc.vector.tensor_sub(
                            hi[:, CH * k:CH * k + CH],
                            xt[o + 1][:, CH * k:CH * k + CH], po[k][:])
                    nc.sync.dma_start(low_d[128 * o:128 * o + 128, :], lo[:])
                    nc.sync.dma_start(high_d[128 * o:128 * o + 128, :], hi[:])

    nc.compile()
    return nc


_CACHE = {}


def _get_plan_nc():
    if "pn" not in _CACHE:
        plan = _build_plan()
        nc = _build_program(plan)
        _CACHE["pn"] = (plan, nc)
    return _CACHE["pn"]


def _make_in_maps(plan, x):
    x = np.asarray(x)
    B, Fd, L_ = x.shape
    xs = np.ascontiguousarray(
        x.reshape(B * Fd, L_).T).astype(BF16_NP)   # sig layout [L, rows]
    in_maps = []
    for c in range(N_CORES):
        xbase = 2048 * c + X_OFF
        xloc = np.zeros((NPB * 128, ROWS), BF16_NP)
        lo_ = max(xbase, 0)
        hi_ = min(xbase + NPB * 128, L_)
        xloc[lo_ - xbase:hi_ - xbase] = xs[lo_:hi_]
        in_maps.append({"x": xloc, "w1": plan["w1"][c], "w2": plan["w2"][c]})
    return in_maps


def kernel(x):
    x = np.asarray(x)
    B, Fd, L_ = x.shape
    in_dtype = x.dtype
    plan, nc = _get_plan_nc()
    in_maps = _make_in_maps(plan, x)
    res = run_bass_kernel_spmd(nc, in_maps, list(range(N_CORES)))
    low_sig = np.concatenate([np.asarray(r["low"]) for r in res.results], axis=0)
    high_sig = np.concatenate([np.asarray(r["high"]) for r in res.results], axis=0)
    low = np.ascontiguousarray(low_sig.T).astype(np.float32).reshape(B, Fd, L_)
    high = np.ascontiguousarray(high_sig.T).astype(np.float32).reshape(B, Fd, L_)
    return low.astype(in_dtype, copy=False), high.astype(in_dtype, copy=False)
